# revision 11
# baseline (speedup 1.0000x reference)
"""Top-1 MoE mapper kernel for Trainium2, SPMD over 8 NeuronCores (v3).

Problem (hardcoded shapes):
  x  [2048, 1, 1024] f32   token inputs
  t  [2048, 8, 4096] f32   gating context
  W  [12, 1024, 4096] f32  expert weights
  b  [12, 4096] f32        expert biases
  Wg [4096, 12] f32        gate weights
  bg [12] f32              gate bias
  out[b] = x[b] @ W[argmax(t[b].mean(T) @ Wg + bg)] + b[...]  -> [2048, 1, 4096]

v3 strategy:
  - Gating exact f32, data-parallel over B. t chunks alternate between the
    Activation and SP DMA queues (a single queue saturates ~200GB/s; two
    reach ~330GB/s). W does NOT load during gating at all.
  - Expert path in bf16 (host-cast W/x, bf16 out upcast on host).
  - Capacity 112 per (expert, token-half): slot(tok) = e*224 + half*112 +
    rank-within-half. 2688 slots = exactly 21 slot tiles, and each half's
    routing + x-row scatters run right after ITS AllGather: half 0's
    dispatch overlaps AG1 / the tail of gating.
  - Token dispatch is a direct indirect SCATTER of preloaded x rows into
    per-half x_slots DRAM regions (no slot->token table, no indirect
    gathers): the expert phase does plain contiguous loads + PE transposes.
  - Output-transposed expert matmuls (out^T[col, slot]): lhsT = W tile
    (streamed 3-deep during the expert phase), rhs = xgT expert block,
    bias fused into the Activation-engine PSUM evacuation.
"""

import numpy as np
import ml_dtypes

import concourse.bass as bass
import concourse.bacc as bacc
import concourse.mybir as mybir
import concourse.tile as tile
from concourse.bass import IndirectOffsetOnAxis
from concourse.bass_utils import run_bass_kernel_spmd

F32 = mybir.dt.float32
BF16 = mybir.dt.bfloat16
U32 = mybir.dt.uint32

B, T, IN, OUT, E = 2048, 8, 1024, 4096, 12
NCORES = 8
BS = B // NCORES            # 256 tokens per core (gating shard)
CS = OUT // NCORES          # 512 output columns per core (expert shard)
CAPH = 112                  # capacity slots per expert per token-half
CAPT = 2 * CAPH             # 224 slots per expert
SLOTS = E * CAPT            # 2688 = 21 * 128
NT = B // 128               # 16 token tiles globally
NTT = BS // 128             # 2 token tiles per core
NTH = NT // 2               # 8 token tiles per half
HSLOTS = E * CAPH           # 1344 slots per half
NJH = (HSLOTS + 127) // 128  # 11 slot tiles per half (1408 padded)
NKX = IN // 128             # 8 k-tiles over the expert contraction
NMT = CS // 128             # 4 column tiles per core


def build_kernel(enable_asserts: bool = False):
    nc = bacc.Bacc(
        "TRN2",
        target_bir_lowering=False,
        debug=False,
        enable_asserts=enable_asserts,
        num_devices=NCORES,
    )

    # ---- I/O -------------------------------------------------------------
    t_sh = nc.dram_tensor("t_sh", [BS, T, OUT], F32, kind="ExternalInput")
    x_bf = nc.dram_tensor("x_bf", [B, IN], BF16, kind="ExternalInput")
    w_sh = nc.dram_tensor("w_sh", [E, IN, CS], BF16, kind="ExternalInput")
    b_cols = nc.dram_tensor("b_cols", [128, E * NMT], F32, kind="ExternalInput")
    wg_s = nc.dram_tensor("wg_s", [OUT, E], F32, kind="ExternalInput")  # Wg/T
    bg_r = nc.dram_tensor("bg_r", [1, E], F32, kind="ExternalInput")
    ident = nc.dram_tensor("ident", [128, 128], F32, kind="ExternalInput")
    identb = nc.dram_tensor("identb", [128, 128], BF16, kind="ExternalInput")
    lsl = nc.dram_tensor("lsl", [128, 128], F32, kind="ExternalInput")
    colsel8 = nc.dram_tensor("colsel8", [NTH, NTH * 128], F32, kind="ExternalInput")
    iota_e = nc.dram_tensor("iota_e", [128, E], F32, kind="ExternalInput")

    out_T = nc.dram_tensor("out_T", [CS, SLOTS], BF16, kind="ExternalOutput")
    top1_out = nc.dram_tensor("top1_out", [B, 1], U32, kind="ExternalOutput")

    with tile.TileContext(nc) as tc:
        with (
            tc.tile_pool(name="consts", bufs=1) as cpool,
            tc.tile_pool(name="dram", bufs=1, space="DRAM") as dpool,
            tc.tile_pool(name="wp", bufs=3) as wpool,
            tc.tile_pool(name="gat", bufs=2) as gpool,
            tc.tile_pool(name="gats", bufs=2) as gspool,
            tc.tile_pool(name="xs", bufs=1) as xspool,
            tc.tile_pool(name="xg", bufs=4) as xgpool,
            tc.tile_pool(name="rout", bufs=1) as rpool,
            tc.tile_pool(name="op", bufs=3) as opool,
            tc.tile_pool(name="tp", bufs=4, space="PSUM") as tpool,
            tc.tile_pool(name="gps", bufs=2, space="PSUM") as gpsum,
            tc.tile_pool(name="ops", bufs=2, space="PSUM") as opsum,
        ):
            # ---- constants (sync queue, tiny, first) ---------------------
            ident_sb = cpool.tile([128, 128], F32)
            nc.sync.dma_start(ident_sb[:], ident[:, :])
            identb_sb = cpool.tile([128, 128], BF16)
            nc.sync.dma_start(identb_sb[:], identb[:, :])
            lsl_sb = cpool.tile([128, 128], F32)
            nc.sync.dma_start(lsl_sb[:], lsl[:, :])
            colsel_sb = cpool.tile([NTH, NTH * 128], F32)
            nc.sync.dma_start(colsel_sb[:], colsel8[:, :])
            iota_e_sb = cpool.tile([128, E], F32)
            nc.sync.dma_start(iota_e_sb[:], iota_e[:, :])
            ones_sb = cpool.tile([128, 128], F32)
            nc.vector.memset(ones_sb[:], 1.0)
            wg_sb = cpool.tile([128, (OUT // 128) * E], F32)
            nc.sync.dma_start(
                wg_sb[:].rearrange("p (k e) -> p k e", e=E),
                wg_s[:, :].rearrange("(k p) e -> p k e", p=128),
            )
            bg_sb = cpool.tile([1, E], F32)
            nc.sync.dma_start(bg_sb[:], bg_r[:, :])
            b_sb = cpool.tile([128, E * NMT], F32)
            nc.sync.dma_start(b_sb[:], b_cols[:, :])

            # DRAM scratch
            top1_loc = [dpool.tile([128, 1], U32, name=f"t1l{i}") for i in range(NTT)]
            halves = [
                dpool.tile([NCORES * 128, 1], U32, name=f"half{i}") for i in range(NTT)
            ]
            x_slots = [
                dpool.tile([NJH * 128, IN], BF16, name=f"xsl{h}") for h in range(NTT)
            ]

            # x rows resident in SBUF (token order), for the dispatch scatter.
            # Loaded on the sync queue AFTER the odd t chunks (during the
            # AllGather window) so it doesn't compete with gating DMA.
            x_sb = xspool.tile([128, NT, IN], BF16)

            # transposed gathered activations: [k-partition, kt, slot]
            # unified layout: expert e -> cols [e*224, e*224+112) half 0,
            # [e*224+112, (e+1)*224) half 1.
            xgT_all = xspool.tile([128, NKX, SLOTS], BF16)

            # ================= phase 1+2: gating & overlapped allgather ===
            DC = 1024         # d-chunk width
            NDC = OUT // DC
            mxis = []
            for tt in range(NTT):
                gps = gpsum.tile([E, 128], F32, tag="gps")
                for dc in range(NDC):
                    chunk = gpool.tile([128, T, DC], F32, tag="tchunk")
                    q = nc.scalar if (tt * NDC + dc) % 2 == 0 else nc.sync
                    q.dma_start(
                        chunk[:],
                        t_sh[tt * 128 : (tt + 1) * 128, :, dc * DC : (dc + 1) * DC],
                    )
                    cf = chunk[:].rearrange("p t d -> p (t d)")
                    nc.vector.tensor_add(
                        cf[:, 0 : 4 * DC], cf[:, 0 : 4 * DC], cf[:, 4 * DC : 8 * DC]
                    )
                    nc.vector.tensor_add(
                        cf[:, 0 : 2 * DC], cf[:, 0 : 2 * DC], cf[:, 2 * DC : 4 * DC]
                    )
                    nc.vector.tensor_add(
                        cf[:, 0:DC], cf[:, 0:DC], cf[:, DC : 2 * DC]
                    )
                    for k in range(DC // 128):
                        kt = dc * (DC // 128) + k
                        ptr = tpool.tile([128, 128], F32, tag="tp")
                        nc.tensor.transpose(
                            ptr[:],
                            chunk[:, 0, k * 128 : (k + 1) * 128],
                            ident_sb[:, :],
                        )
                        tst = gspool.tile([128, 128], F32, tag="tsT", bufs=6)
                        nc.vector.tensor_copy(tst[:], ptr[:])
                        nc.tensor.matmul(
                            gps[:],
                            lhsT=wg_sb[:, kt * E : (kt + 1) * E],
                            rhs=tst[:],
                            start=(kt == 0),
                            stop=False,
                        )
                nc.tensor.matmul(
                    gps[:],
                    lhsT=bg_sb[0:1, :],
                    rhs=ones_sb[0:1, 0:128],
                    start=False,
                    stop=True,
                )
                gT_sb = gspool.tile([E, 128], F32, tag="gT")
                nc.vector.tensor_copy(gT_sb[:], gps[:])
                gp = tpool.tile([128, E], F32, tag="tp")
                nc.tensor.transpose(gp[:], gT_sb[:], ident_sb[0:E, 0:E])
                gate_sb = gspool.tile([128, E], F32, tag="gate")
                nc.vector.tensor_copy(gate_sb[:], gp[:])
                mxv = gspool.tile([128, 8], F32, tag="mxv")
                mxi = gspool.tile([128, 8], U32, tag="mxi")
                nc.vector.max_with_indices(mxv[:], mxi[:], gate_sb[:])
                mxis.append(mxi)
                nc.gpsimd.dma_start(top1_loc[tt][:, :], mxi[:, 0:1])
                nc.gpsimd.collective_compute(
                    "AllGather",
                    mybir.AluOpType.bypass,
                    replica_groups=[list(range(NCORES))],
                    ins=[top1_loc[tt][:].opt()],
                    outs=[halves[tt][:].opt()],
                )
                nc.sync.dma_start(
                    top1_out[:, :].rearrange("(c t p) one -> t c p one", t=NTT, p=128)[tt],
                    halves[tt][:].rearrange("(c p) one -> c p one", p=128),
                )

            # x preload: even-half rows first (needed by half-0 scatters)
            for h in range(NTT):
                nc.sync.dma_start(
                    x_sb[:, h::NTT, :],
                    x_bf[:, :].rearrange("(i p) d -> p i d", p=128)[:, h::NTT, :],
                )

            # ---- W stream (sync queue, after x; bufs=3 throttles) --------
            wts = []
            for e in range(3):
                wt = wpool.tile([128, NKX, CS], BF16, tag="wt", name=f"wt{e}")
                nc.sync.dma_start(
                    wt[:], w_sh[e].rearrange("(k p) n -> p k n", p=128)
                )
                wts.append(wt)

            # ================= phase 3: per-half routing + dispatch =======
            # half h covers global token tiles {2c+h}; halves[h] rows (c,p)
            # hold top1 of global token (2c+h)*128+p.
            for h in range(NTT):
                tb = rpool.tile([128, NTH], U32, name=f"tb{h}")
                nc.scalar.dma_start(
                    tb[:], halves[h][:].rearrange("(c p) one -> p c one", p=128)
                )
                t1f = rpool.tile([128, NTH], F32, name=f"t1f{h}")
                nc.vector.tensor_copy(t1f[:], tb[:])
                oh = rpool.tile([128, NTH * E], F32, name=f"oh{h}")
                for i in range(NTH):
                    nc.vector.tensor_tensor(
                        out=oh[:, i * E : (i + 1) * E],
                        in0=t1f[:, i : i + 1].to_broadcast([128, E]),
                        in1=iota_e_sb[:],
                        op=mybir.AluOpType.is_equal,
                    )
                pcnt = tpool.tile([1, NTH * E], F32, tag="tp")
                nc.tensor.matmul(
                    pcnt[:], lhsT=ones_sb[0:128, 0:1], rhs=oh[:],
                    start=True, stop=True,
                )
                cnt_sb = rpool.tile([1, NTH * E], F32, name=f"cnt{h}")
                nc.vector.tensor_copy(cnt_sb[:], pcnt[:])
                pc2 = tpool.tile([NTH, E], F32, tag="tp")
                for e in range(E):
                    nc.tensor.transpose(
                        pc2[:, e : e + 1],
                        cnt_sb[0:1, :].rearrange("one (i e) -> one i e", e=E)[:, :, e],
                        ident_sb[0:1, 0:1],
                    )
                c2_sb = rpool.tile([NTH, E], F32, name=f"c2{h}")
                nc.vector.tensor_copy(c2_sb[:], pc2[:])

                pr = tpool.tile([128, NTH * E], F32, tag="tp")
                nc.tensor.matmul(
                    pr[:], lhsT=lsl_sb[:], rhs=oh[:],
                    start=True, stop=False, skip_group_check=True,
                )
                for i in range(NTH):
                    nc.tensor.matmul(
                        pr[:, i * E : (i + 1) * E],
                        lhsT=colsel_sb[:, i * 128 : (i + 1) * 128],
                        rhs=c2_sb[:],
                        start=False, stop=True, skip_group_check=True,
                    )
                sel = rpool.tile([128, NTH * E], F32, name=f"sel{h}")
                nc.vector.tensor_mul(sel[:], pr[:], oh[:])
                rank3 = rpool.tile([128, NTH, 1], F32, name=f"rank{h}")
                nc.vector.reduce_sum(
                    rank3[:],
                    sel[:].rearrange("p (i e) -> p i e", e=E),
                    axis=mybir.AxisListType.X,
                )
                posf = rpool.tile([128, NTH], F32, name=f"posf{h}")
                nc.vector.tensor_scalar(
                    posf[:], t1f[:], float(CAPH), scalar2=None,
                    op0=mybir.AluOpType.mult,
                )
                nc.vector.tensor_add(posf[:], posf[:], rank3[:, :, 0])
                posu = rpool.tile([128, NTH], U32, name=f"posu{h}")
                nc.vector.tensor_copy(posu[:], posf[:])
                # dispatch: scatter x rows into this half's slot space
                for i in range(NTH):
                    gi = 2 * i + h  # global token tile
                    nc.gpsimd.indirect_dma_start(
                        out=x_slots[h][:, :],
                        out_offset=IndirectOffsetOnAxis(ap=posu[:, i : i + 1], axis=0),
                        in_=x_sb[:, gi, :],
                        in_offset=None,
                        bounds_check=NJH * 128 - 1,
                        oob_is_err=False,
                    )

            # ================= phase 4: load + transpose + expert matmul ==
            # half-h slot tile j (slots [j*128,(j+1)*128) of half h) maps to
            # unified xgT cols e*CAPT + h*CAPH + (s - e*CAPH) for s in tile.
            def xgt_segments(h, j):
                """Unified-layout column segments for half h's slot range
                [j*128, (j+1)*128). Returns [(src_off, dst_col, width)]."""
                segs = []
                s = j * 128
                end = min((j + 1) * 128, HSLOTS)
                while s < end:
                    e = s // CAPH
                    seg_end = min(end, (e + 1) * CAPH)
                    dst = e * CAPT + h * CAPH + (s - e * CAPH)
                    segs.append((s - j * 128, dst, seg_end - s))
                    s = seg_end
                return segs

            # experts become ready once both halves' tiles covering their
            # slot range are transposed; interleave loads so half 0 streams
            # first (its scatters finish during AG1), then half 1.
            done_tiles = set()
            emitted = set()

            def emit_ready_experts():
                for e in range(E):
                    if e in emitted:
                        continue
                    need = set()
                    for h in range(NTT):
                        lo = e * CAPH // 128
                        hi = ((e + 1) * CAPH - 1) // 128
                        need.update((h, j) for j in range(lo, hi + 1))
                    if not need <= done_tiles:
                        continue
                    emitted.add(e)
                    wt = wts[e]
                    s0 = e * CAPT
                    for mt in range(NMT):
                        po = opsum.tile([128, CAPT], F32, tag="po")
                        for kt in range(NKX):
                            nc.tensor.matmul(
                                po[:],
                                lhsT=wt[:, kt, mt * 128 : (mt + 1) * 128],
                                rhs=xgT_all[:, kt, s0 : s0 + CAPT],
                                start=(kt == 0),
                                stop=(kt == NKX - 1),
                            )
                        ot = opool.tile([128, CAPT], BF16, tag="ot")
                        nc.scalar.activation(
                            ot[:], po[:],
                            mybir.ActivationFunctionType.Identity,
                            bias=b_sb[:, e * NMT + mt : e * NMT + mt + 1],
                            scale=1.0,
                        )
                        nc.sync.dma_start(
                            out_T[mt * 128 : (mt + 1) * 128, s0 : s0 + CAPT], ot[:]
                        )

            next_w = 3
            copy_i = 0
            order = [(0, j) for j in range(NJH)] + [(1, j) for j in range(NJH)]
            for n, (h, j) in enumerate(order):
                xg = xgpool.tile([128, IN], BF16, tag="xg")
                q = nc.scalar if n % 2 == 0 else nc.sync
                q.dma_start(xg[:], x_slots[h][j * 128 : (j + 1) * 128, :])
                segs = xgt_segments(h, j)
                for kt in range(NKX):
                    tp = tpool.tile([128, 128], BF16, tag="tp")
                    nc.tensor.transpose(
                        tp[:], xg[:, kt * 128 : (kt + 1) * 128], identb_sb[:, :]
                    )
                    for src, dst, width in segs:
                        if copy_i % 2 == 0:
                            nc.vector.tensor_copy(
                                xgT_all[:, kt, dst : dst + width],
                                tp[:, src : src + width],
                            )
                        else:
                            nc.scalar.copy(
                                xgT_all[:, kt, dst : dst + width],
                                tp[:, src : src + width],
                            )
                        copy_i += 1
                done_tiles.add((h, j))
                # stream upcoming expert weights behind the pipeline
                if n % 2 == 1 and next_w < E:
                    wt = wpool.tile([128, NKX, CS], BF16, tag="wt", name=f"wt{next_w}")
                    nc.sync.dma_start(
                        wt[:], w_sh[next_w].rearrange("(k p) n -> p k n", p=128)
                    )
                    wts.append(wt)
                    next_w += 1
                emit_ready_experts()
            assert len(emitted) == E, emitted

    nc.compile()
    return nc


def make_in_maps(inputs: dict) -> list[dict]:
    x = np.ascontiguousarray(np.asarray(inputs["x"], dtype=np.float32))
    t = np.ascontiguousarray(np.asarray(inputs["t"], dtype=np.float32))
    W = np.ascontiguousarray(np.asarray(inputs["W"], dtype=np.float32))
    b = np.ascontiguousarray(np.asarray(inputs["b"], dtype=np.float32))
    Wg = np.ascontiguousarray(np.asarray(inputs["Wg"], dtype=np.float32))
    bg = np.ascontiguousarray(np.asarray(inputs["bg"], dtype=np.float32))

    x_bf = np.ascontiguousarray(x[:, 0, :]).astype(ml_dtypes.bfloat16)
    ident = np.eye(128, dtype=np.float32)
    identb = np.eye(128, dtype=np.float32).astype(ml_dtypes.bfloat16)
    lsl = np.triu(np.ones((128, 128), np.float32), k=1)  # lsl[r,c]=1 iff r<c
    # colsel8[j, c*128+m] = 1 iff j < c (per-half tile-base prefix selector)
    colsel8 = np.zeros((NTH, NTH * 128), np.float32)
    for c in range(NTH):
        colsel8[:c, c * 128 : (c + 1) * 128] = 1.0
    iota_e = np.tile(np.arange(E, dtype=np.float32)[None, :], (128, 1))

    in_maps = []
    for c in range(NCORES):
        cs = slice(c * CS, (c + 1) * CS)
        b_cols = np.ascontiguousarray(
            b[:, cs].reshape(E, NMT, 128).transpose(2, 0, 1).reshape(128, E * NMT)
        )
        in_maps.append({
            "t_sh": np.ascontiguousarray(t[c * BS : (c + 1) * BS]),
            "x_bf": x_bf,
            "w_sh": np.ascontiguousarray(W[:, :, cs]).astype(ml_dtypes.bfloat16),
            "b_cols": b_cols,
            "wg_s": np.ascontiguousarray(Wg / float(T)),
            "bg_r": bg.reshape(1, E),
            "ident": ident,
            "identb": identb,
            "lsl": lsl,
            "colsel8": colsel8,
            "iota_e": iota_e,
        })
    return in_maps


def assemble_output(per_core_results: list[dict]) -> np.ndarray:
    top1 = np.asarray(per_core_results[0]["top1_out"]).reshape(B).astype(np.int64)
    # recompute slot(token) exactly as the device did: per (expert, half)
    # rank in global token order; half = (token//128) % 2
    counts = np.zeros((E, 2), dtype=np.int64)
    slot = np.zeros(B, dtype=np.int64)
    for i in range(B):
        e = top1[i]
        h = (i // 128) % 2
        slot[i] = e * CAPT + h * CAPH + counts[e, h]
        counts[e, h] += 1
    assert counts.max() <= CAPH, f"expert-half overflow: {counts}"
    out = np.empty((B, 1, OUT), dtype=np.float32)
    for c in range(NCORES):
        osl = np.asarray(per_core_results[c]["out_T"]).astype(np.float32)
        out[:, 0, c * CS : (c + 1) * CS] = osl[:, slot].T
    return out


_NC_CACHE = {}


def kernel(**inputs) -> np.ndarray:
    if "nc" not in _NC_CACHE:
        _NC_CACHE["nc"] = build_kernel()
    nc = _NC_CACHE["nc"]
    in_maps = make_in_maps(inputs)
    res = run_bass_kernel_spmd(nc, in_maps, core_ids=list(range(NCORES)))
    return assemble_output(res.results)
